# revision 1
# baseline (speedup 1.0000x reference)
"""GNN edge-softmax attention kernel for 8 Trainium2 NeuronCores.

Math: logits = src@(W_src@a) + dest@(W_dest@a) + ea@(W_edge@a)   [E]
      s = leaky_relu(logits, 0.2); val = exp(s)
      out[e] = val[e] / (segsum[col[e]] + eps)     (softmax over dest node)

Strategy:
  * Fold the three projection matrices with the attention vector on host ->
    three matvecs; the kernel is memory-bound streaming of src/dest/ea.
  * Host sorts edges by destination node; core c owns a contiguous node
    range so every softmax segment is core-local (no collectives).
  * Per core (compiled per-core since window offsets are data-dependent):
      phase 1: PE matvec over transposed input chunks -> logits in PSUM
               [1,n] rows -> ACT copy -> DMA to DRAM scratch.
      phase 1.5: reload scratch as [128, T] (edge-per-partition), leaky+exp.
      phase 2: per 128-edge tile build one-hot (iota == wloc) on DVE, then
               PE matmul OH^T @ val scatters windowed segment sums into PSUM.
      phase 2.5: 1/(sum+eps) (DVE reciprocal), replicate table to all
               partitions via doubling DMAs.
      phase 3: per tile TensorTensorReduce(OH * inv_window) -> gathered
               1/segsum per edge; multiply by val; DMA out.
"""

import math
import os
import sys
import threading
import time

import numpy as np

sys.path.insert(0, "/opt/trn_rl_repo")

P = 128
NCORES = 8
NEG_SLOPE = 0.2
EPS = 1e-16
CH_NODES = 800   # nodes per chunk; multiple of 32
BLK = 1536       # phase-1 block (3 PSUM banks of 512 fp32)
PAD_W = 255.0    # wloc marker for pad slots (never matches iota < W)

LAST_EXEC_NS = None
LAST_WALL_NS = None

_PROGRAM_CACHE = {}


# --------------------------------------------------------------------------- #
# Host-side preparation
# --------------------------------------------------------------------------- #

def _ceil_to(x, m):
    return (x + m - 1) // m * m


def _prep_core(core_id, node_lo, node_hi, col_sorted, e_lo, e_hi):
    """Compute chunk/tile metadata for one core.

    Returns dict with slot structure. Slots = sorted real edges per chunk,
    each chunk padded to a multiple of 128.
    """
    chunks = []
    n_nodes_core = node_hi - node_lo
    n_chunks = max(1, math.ceil(n_nodes_core / CH_NODES))
    slot0 = 0
    max_need = 0
    for ci in range(n_chunks):
        nb = node_lo + ci * CH_NODES
        ne = min(node_hi, nb + CH_NODES)
        ce_lo = np.searchsorted(col_sorted, nb, side="left")
        ce_hi = np.searchsorted(col_sorted, ne, side="left")
        ce_lo = max(ce_lo, e_lo)
        ce_hi = min(ce_hi, e_hi)
        n_real = int(ce_hi - ce_lo)
        S_c = max(P, _ceil_to(n_real, P)) if n_real > 0 else 0
        if S_c == 0:
            chunks.append(dict(nb=nb, ne=ne, e_lo=int(ce_lo), e_hi=int(ce_hi),
                               S=0, T=0, slot0=slot0, tiles=[]))
            continue
        T_c = S_c // P
        lcol = (col_sorted[ce_lo:ce_hi] - nb).astype(np.int64)
        tiles = []
        for t in range(T_c):
            s0 = t * P
            s1 = min(n_real, s0 + P)
            if s1 <= s0:
                tiles.append((0, None))  # pad-only tile
                continue
            seg = lcol[s0:s1]
            a0 = int(seg[0] // 64) * 64   # matmul out base partition: 0/64 only
            need = int(seg[-1]) - a0 + 1
            max_need = max(max_need, need)
            tiles.append((a0, (s0, s1)))
        chunks.append(dict(nb=nb, ne=ne, e_lo=int(ce_lo), e_hi=int(ce_hi),
                           S=S_c, T=T_c, slot0=slot0, tiles=tiles, lcol=lcol))
        slot0 += S_c
    return dict(core=core_id, chunks=chunks, S_total=slot0,
                T_total=slot0 // P, max_need=max_need)


def _host_prep(src, dest, edge_attr, col, n_nodes):
    E = src.shape[0]
    npc = math.ceil(n_nodes / NCORES)
    perm = np.argsort(col, kind="stable")
    col_s = col[perm]

    metas = []
    max_need = 0
    for c in range(NCORES):
        node_lo = c * npc
        node_hi = min(n_nodes, (c + 1) * npc)
        if node_lo >= n_nodes:
            node_lo = node_hi = n_nodes
        e_lo = int(np.searchsorted(col_s, node_lo, side="left"))
        e_hi = int(np.searchsorted(col_s, node_hi, side="left"))
        m = _prep_core(c, node_lo, node_hi, col_s, e_lo, e_hi)
        metas.append(m)
        max_need = max(max_need, m["max_need"])

    W = 128  # max per-tile window width (allocation/iota size)
    assert max_need <= 128, f"tile node-span {max_need} > 128 unsupported"

    # finalize per-tile segments; width adapts per tile (64/96/128)
    cap_cols = _ceil_to(CH_NODES + W, P) // P
    for m in metas:
        for ch in m["chunks"]:
            # recompute per-tile need to pick width
            segs_per_tile = []
            for (a0, rng) in ch["tiles"]:
                if rng is None:
                    wt = 64
                else:
                    s0, s1 = rng
                    need = int(ch["lcol"][s1 - 1]) - a0 + 1
                    wt = 64 if need <= 64 else (96 if need <= 96 else 128)
                segs = []
                w = 0
                while w < wt:
                    n0 = a0 + w
                    p0 = n0 % P
                    assert p0 in (0, 64)
                    run = min(wt - w, (P if p0 == 0 else 64))
                    segs.append((w, w + run, p0, n0 // P))
                    w += run
                segs_per_tile.append((a0, wt, segs))
            ch["tile_segs"] = segs_per_tile

    # build per-core arrays
    per_core = []
    for m in metas:
        S = m["S_total"]
        if S == 0:
            per_core.append(None)
            continue
        srcT = np.zeros((P, S), np.float32)
        destT = np.zeros((P, S), np.float32)
        eaT = np.zeros((edge_attr.shape[1], S), np.float32)
        wloc = np.full((S,), PAD_W, np.float32)
        oidx = np.full((S,), -1, np.int64)
        for ch in m["chunks"]:
            if ch["S"] == 0:
                continue
            sl0 = ch["slot0"]
            nr = ch["e_hi"] - ch["e_lo"]
            eids = perm[ch["e_lo"]:ch["e_hi"]]
            srcT[:, sl0:sl0 + nr] = src[eids].T
            destT[:, sl0:sl0 + nr] = dest[eids].T
            eaT[:, sl0:sl0 + nr] = edge_attr[eids].T
            oidx[sl0:sl0 + nr] = eids
            lcol = ch["lcol"]
            wl = np.full((ch["S"],), PAD_W, np.float32)
            for t, (a0, rng) in enumerate(ch["tiles"]):
                if rng is None:
                    continue
                s0, s1 = rng
                wl[s0:s1] = (lcol[s0:s1] - a0).astype(np.float32)
            wloc[sl0:sl0 + ch["S"]] = wl
        # wloc in [128, T] (partition, tile) layout
        wlocf = np.ascontiguousarray(wloc.reshape(-1, P).T)
        per_core.append(dict(srcT=srcT, destT=destT, eaT=eaT,
                             wlocf=wlocf, oidx=oidx))
    return metas, per_core, W, cap_cols


# --------------------------------------------------------------------------- #
# Device program builder (one per core)
# --------------------------------------------------------------------------- #

def _build_core_program(meta, W, cap_cols, IN, ED, stop_phase=3):
    from concourse import bacc, bass, dve_ops, mybir
    from concourse import tile

    S_total = meta["S_total"]
    T_total = meta["T_total"]
    assert S_total > 0
    f32 = mybir.dt.float32
    ncap = cap_cols * P

    nc = bacc.Bacc("TRN2", target_bir_lowering=False, debug=True)

    xsrcT = nc.declare_dram_parameter("xsrcT", [P, S_total], f32, isOutput=False)
    xdestT = nc.declare_dram_parameter("xdestT", [P, S_total], f32, isOutput=False)
    xeaT = nc.declare_dram_parameter("xeaT", [ED, S_total], f32, isOutput=False)
    xwloc = nc.declare_dram_parameter("xwloc", [P, T_total], f32, isOutput=False)
    xvs = nc.declare_dram_parameter("xvs", [IN, 1], f32, isOutput=False)
    xvd = nc.declare_dram_parameter("xvd", [IN, 1], f32, isOutput=False)
    xve = nc.declare_dram_parameter("xve", [ED, 1], f32, isOutput=False)
    xiota = nc.declare_dram_parameter("xiota", [P, W], f32, isOutput=False)
    yout = nc.declare_dram_parameter("yout", [P, T_total], f32, isOutput=True)

    T_max = max((ch["T"] for ch in meta["chunks"]), default=1)
    S_max = T_max * P

    AF = mybir.ActivationFunctionType
    OP = mybir.AluOpType

    with tile.TileContext(nc) as tc:
        with (
            tc.tile_pool(name="consts", bufs=1) as cpool,
            tc.tile_pool(name="stream", bufs=3) as spool,
            tc.tile_pool(name="chunkbuf", bufs=2) as kpool,
            tc.tile_pool(name="ohbuf", bufs=4) as opool,
            tc.tile_pool(name="ps_s", bufs=2, space="PSUM") as ps_pool,
            tc.tile_pool(name="ps_g", bufs=2, space="PSUM") as pg_pool,
            tc.tile_pool(name="dram", bufs=2, space="DRAM") as dpool,
        ):
            vs = cpool.tile([IN, 1], f32, tag="vs")
            vd = cpool.tile([IN, 1], f32, tag="vd")
            ve = cpool.tile([ED, 1], f32, tag="ve")
            iota = cpool.tile([P, W], f32, tag="iota")
            zer = cpool.tile([P, P], f32, tag="zer")
            nc.sync.dma_start(out=vs[:], in_=xvs[:])
            nc.sync.dma_start(out=vd[:], in_=xvd[:])
            nc.sync.dma_start(out=ve[:], in_=xve[:])
            nc.sync.dma_start(out=iota[:], in_=xiota[:])
            nc.vector.memset(zer[:], 0.0)

            for ch in meta["chunks"]:
                S_c, T_c = ch["S"], ch["T"]
                if S_c == 0:
                    continue
                sl0 = ch["slot0"]
                t0 = sl0 // P

                # ---- phase 1: logits for this chunk's slots ----
                s_dram = dpool.tile([1, S_max], f32, tag="sdram")
                n_blk = math.ceil(S_c / BLK)
                for b in range(n_blk):
                    o = b * BLK
                    n = min(BLK, S_c - o)
                    bsrc = spool.tile([P, BLK], f32, tag="bsrc")
                    bdst = spool.tile([P, BLK], f32, tag="bdst")
                    bea = spool.tile([ED, BLK], f32, tag="bea")
                    nc.sync.dma_start(out=bsrc[:, :n], in_=xsrcT[:, sl0 + o: sl0 + o + n])
                    nc.sync.dma_start(out=bdst[:, :n], in_=xdestT[:, sl0 + o: sl0 + o + n])
                    nc.sync.dma_start(out=bea[:, :n], in_=xeaT[:, sl0 + o: sl0 + o + n])
                    ps = ps_pool.tile([1, BLK], f32, tag="ps_s")
                    for j in range(math.ceil(n / 512)):
                        jo = j * 512
                        jn = min(512, n - jo)
                        nc.tensor.matmul(out=ps[0:1, jo:jo + jn],
                                         lhsT=vs[:, :], rhs=bsrc[:, jo:jo + jn],
                                         start=True, stop=False)
                        nc.tensor.matmul(out=ps[0:1, jo:jo + jn],
                                         lhsT=vd[:, :], rhs=bdst[:, jo:jo + jn],
                                         start=False, stop=False)
                        nc.tensor.matmul(out=ps[0:1, jo:jo + jn],
                                         lhsT=ve[:, :], rhs=bea[:, jo:jo + jn],
                                         start=False, stop=True)
                    srow = spool.tile([1, BLK], f32, tag="srow")
                    nc.scalar.activation(srow[0:1, :n], ps[0:1, :n], AF.Copy)
                    nc.sync.dma_start(out=s_dram[0:1, o:o + n], in_=srow[0:1, :n])

                if stop_phase == 1:
                    continue
                # ---- phase 1.5: reload as [128, T_c]; leaky relu + exp ----
                val = kpool.tile([P, T_max], f32, tag="val")
                tmp = kpool.tile([P, T_max], f32, tag="tmp")
                nc.sync.dma_start(
                    out=val[:, :T_c],
                    in_=s_dram[0, :S_c].rearrange("(t p) -> p t", p=P),
                )
                nc.vector.tensor_scalar(out=tmp[:, :T_c], in0=val[:, :T_c],
                                        scalar1=NEG_SLOPE, scalar2=None,
                                        op0=OP.mult)
                nc.vector.tensor_tensor(out=tmp[:, :T_c], in0=val[:, :T_c],
                                        in1=tmp[:, :T_c], op=OP.max)
                nc.scalar.activation(val[:, :T_c], tmp[:, :T_c], AF.Exp)

                wl = kpool.tile([P, T_max], f32, tag="wl")
                nc.sync.dma_start(out=wl[:, :T_c], in_=xwloc[:, t0:t0 + T_c])

                if stop_phase == 15:
                    nc.sync.dma_start(out=yout[:, t0:t0 + T_c],
                                      in_=val[:, :T_c])
                    continue
                # ---- phase 2: scatter windowed segment sums into PSUM ----
                psg = pg_pool.tile([P, cap_cols], f32, tag="ps_g")
                nc.tensor.matmul(out=psg[:, :], lhsT=zer[:, :],
                                 rhs=zer[:, :cap_cols], start=True, stop=False)
                for t, (a0, wt, segs) in enumerate(ch["tile_segs"]):
                    oh = opool.tile([P, W], f32, tag="oh")
                    nc.vector.tensor_scalar(
                        out=oh[:, :wt], in0=iota[:, :wt],
                        scalar1=wl[:, t:t + 1], scalar2=None,
                        op0=OP.is_equal,
                    )
                    for (ws, we, p0, f0) in segs:
                        nc.tensor.matmul(
                            out=psg[p0:p0 + (we - ws), f0:f0 + 1],
                            lhsT=oh[:, ws:we], rhs=val[:, t:t + 1],
                            start=False, stop=False,
                            tile_position=(0, p0),
                        )
                # close the accumulation group over the full region (adds 0)
                nc.tensor.matmul(out=psg[:, :], lhsT=zer[:, :],
                                 rhs=zer[:, :cap_cols], start=False, stop=True)

                if stop_phase == 2:
                    nc.sync.dma_start(out=yout[:, t0:t0 + T_c],
                                      in_=val[:, :T_c])
                    continue
                # ---- phase 2.5: reciprocal + replicate table ----
                invs = kpool.tile([P, cap_cols], f32, tag="invs")
                nc.scalar.activation(invs[:, :], psg[:, :], AF.Copy)
                nc.vector.tensor_scalar(out=invs[:, :], in0=invs[:, :],
                                        scalar1=EPS, scalar2=None, op0=OP.add)
                nc.vector.reciprocal(invs[:, :], invs[:, :])
                g_dram = dpool.tile([1, ncap], f32, tag="gdram")
                nc.sync.dma_start(
                    out=g_dram[0, :].rearrange("(f p) -> p f", p=P),
                    in_=invs[:, :],
                )
                gb = kpool.tile([P, ncap], f32, tag="gb")
                nc.sync.dma_start(out=gb[0:1, :], in_=g_dram[0:1, :])
                k = 1
                while k < P:
                    nc.sync.dma_start(out=gb[k:2 * k, :], in_=gb[0:k, :])
                    k *= 2

                # ---- phase 3: gather 1/segsum per edge, multiply, store ----
                if stop_phase == 25:
                    nc.sync.dma_start(out=yout[:, t0:t0 + T_c],
                                      in_=val[:, :T_c])
                    continue
                # out[e] = val[e] * sum_w OH[e,w] * inv[a0+w], fused in one
                # custom-DVE op per tile (scale rides s1 per-partition).
                gath = kpool.tile([P, T_max], f32, tag="gath")
                for t, (a0, wt, segs) in enumerate(ch["tile_segs"]):
                    oh2 = opool.tile([P, W], f32, tag="oh2")
                    nc.vector.tensor_scalar(
                        out=oh2[:, :wt], in0=iota[:, :wt],
                        scalar1=wl[:, t:t + 1], scalar2=None,
                        op0=OP.is_equal,
                    )
                    scr = opool.tile([P, W], f32, tag="scr")
                    nc.vector._custom_dve(
                        dve_ops.TENSOR_TENSOR_REDUCE,
                        out=scr[:, :wt], in0=oh2[:, :wt],
                        in1=gb[:, a0:a0 + wt],
                        s0=0.0, s1=val[:, t:t + 1],
                        accum_out=gath[:, t:t + 1],
                    )
                nc.sync.dma_start(out=yout[:, t0:t0 + T_c], in_=gath[:, :T_c])

    nc.compile()
    return nc


# --------------------------------------------------------------------------- #
# Launcher: run per-core programs concurrently on the 8 devices
# --------------------------------------------------------------------------- #

def _make_runner(nc, device):
    import jax
    from concourse import bass2jax, mybir

    bass2jax.install_neuronx_cc_hook()

    in_names, out_names, out_avals, zero_outs = [], [], [], []
    pname = nc.partition_id_tensor.name if nc.partition_id_tensor else None
    for alloc in nc.m.functions[0].allocations:
        if not isinstance(alloc, mybir.MemoryLocationSet):
            continue
        name = alloc.memorylocations[0].name
        if alloc.kind == "ExternalInput":
            if name != pname:
                in_names.append(name)
        elif alloc.kind == "ExternalOutput":
            shape = tuple(alloc.tensor_shape)
            dtype = mybir.dt.np(alloc.dtype)
            out_names.append(name)
            out_avals.append(jax.core.ShapedArray(shape, dtype))
            zero_outs.append(np.zeros(shape, dtype))
    if nc.dbg_addr is not None:
        dbg = nc.dbg_addr.name
    else:
        dbg = None
    n_params = len(in_names)
    n_outs = len(out_names)
    all_in = in_names + out_names
    if pname is not None:
        all_in = all_in + [pname]
    donate = tuple(range(n_params, n_params + n_outs))

    def _body(*args):
        operands = list(args)
        if pname is not None:
            operands.append(bass2jax.partition_id_tensor())
        outs = bass2jax._bass_exec_p.bind(
            *operands,
            out_avals=tuple(out_avals),
            in_names=tuple(all_in),
            out_names=tuple(out_names),
            lowering_input_output_aliases=(),
            sim_require_finite=False,
            sim_require_nnan=False,
            nc=nc,
        )
        return tuple(outs)

    jitted = jax.jit(_body, donate_argnums=donate, keep_unused=True)

    def stage(in_map):
        args = []
        for nm in in_names:
            if dbg is not None and nm == dbg:
                args.append(jax.device_put(np.zeros((1, 2), np.uint32), device))
            else:
                args.append(jax.device_put(np.asarray(in_map[nm]), device))
        return args

    def execute(staged):
        # donated output buffers are consumed per call; restage (tiny)
        outs = jitted(*staged, *[jax.device_put(z, device) for z in zero_outs])
        return outs, out_names

    return stage, execute


def kernel(src, dest, edge_attr, edge_index, n_nodes,
           W_src, W_dest, W_edge, attn_vector):
    global LAST_EXEC_NS, LAST_WALL_NS
    import jax

    src = np.asarray(src, np.float32)
    dest = np.asarray(dest, np.float32)
    edge_attr = np.asarray(edge_attr, np.float32)
    edge_index = np.asarray(edge_index)
    N = int(n_nodes)
    E, IN = src.shape
    ED = edge_attr.shape[1]

    a = np.asarray(attn_vector, np.float32)[0]
    v_src = (np.asarray(W_src, np.float32) @ a).astype(np.float32)
    v_dest = (np.asarray(W_dest, np.float32) @ a).astype(np.float32)
    v_edge = (np.asarray(W_edge, np.float32) @ a).astype(np.float32)

    col = edge_index[1].astype(np.int64)
    metas, per_core, W, cap_cols = _host_prep(src, dest, edge_attr, col, N)

    iota_host = np.broadcast_to(
        np.arange(W, dtype=np.float32)[None, :], (P, W)).copy()

    devices = jax.devices()
    runners = []
    in_maps = []
    live = []
    for c in range(NCORES):
        if per_core[c] is None:
            continue
        key = ("core", c, metas[c]["S_total"], W, cap_cols, IN, ED,
               tuple(tuple((ch["S"],) + tuple(
                   (a0, wt) + tuple(segs)
                   for (a0, wt, segs) in ch["tile_segs"])
                   for ch in [chh]) for chh in metas[c]["chunks"]))
        kh = hash(key)
        if kh not in _PROGRAM_CACHE:
            nc = _build_core_program(metas[c], W, cap_cols, IN, ED)
            _PROGRAM_CACHE[kh] = _make_runner(nc, devices[c % len(devices)])
        runners.append(_PROGRAM_CACHE[kh])
        in_maps.append(dict(
            xsrcT=per_core[c]["srcT"], xdestT=per_core[c]["destT"],
            xeaT=per_core[c]["eaT"], xwloc=per_core[c]["wlocf"],
            xvs=v_src[:, None], xvd=v_dest[:, None], xve=v_edge[:, None],
            xiota=iota_host,
        ))
        live.append(c)

    # stage all inputs onto their devices first (excluded from timing)
    staged = [r[0](m) for r, m in zip(runners, in_maps)]
    for s in staged:
        jax.block_until_ready(s)

    if os.environ.get("KBENCH"):
        # benchmark mode: settle the tunnel, warm the NEFFs, min-of-3
        settle = float(os.environ.get("KBENCH_SETTLE", "20"))
        if settle > 0:
            time.sleep(settle)
        for _ in range(2):
            warm = [r[1](s) for r, s in zip(runners, staged)]
            for outs, _ in warm:
                jax.block_until_ready(outs)
        best = None
        for _ in range(3):
            t0 = time.perf_counter_ns()
            pending = [r[1](s) for r, s in zip(runners, staged)]
            for outs, _ in pending:
                jax.block_until_ready(outs)
            dt = time.perf_counter_ns() - t0
            best = dt if best is None else min(best, dt)
        LAST_WALL_NS = best
    else:
        # single concurrent execution (grading path)
        t0 = time.perf_counter_ns()
        pending = [r[1](s) for r, s in zip(runners, staged)]
        for outs, _ in pending:
            jax.block_until_ready(outs)
        LAST_WALL_NS = time.perf_counter_ns() - t0

    global LAST_RUNNERS, LAST_STAGED
    LAST_RUNNERS, LAST_STAGED = runners, staged

    results = []
    for outs, names in pending:
        results.append({nm: np.asarray(o) for nm, o in zip(names, outs)})

    out_full = np.zeros((E,), np.float32)
    for i, c in enumerate(live):
        y = results[i]["yout"]          # [128, T_total]
        vals = y.T.reshape(-1)          # slot-ordered
        oidx = per_core[c]["oidx"]
        m = oidx >= 0
        out_full[oidx[m]] = vals[m]
    return out_full[:, None]



# revision 4
# speedup vs baseline: 1600.4433x; 1600.4433x over previous
"""GNN edge segment-softmax attention kernel for 8 Trainium2 NeuronCores.

Math: logits = src@(W_src@a) + dest@(W_dest@a) + ea@(W_edge@a)    [E]
      s = leaky_relu(logits, 0.2); val = exp(s)
      out[e] = val[e] / segsum[col[e]]      (softmax over dest node; the
      reference's eps=1e-16 is negligible: every segsum >= exp(-|s|max) >> eps)

Strategy (single SPMD program on 8 cores, memory-roofline streaming):
  * Fold projection matrices with the attention vector on host: per-edge
    work becomes one 288-wide dot product.
  * Host sorts edges by destination node; core c owns nodes
    [c*N/8, (c+1)*N/8) so every softmax segment is core-local.
  * Within a core, sorted edges are split into 128 partition-strips, each
    strip aligned to segment boundaries (a node's edges never cross strips).
    Slot (p, t): partition p, position t; host gathers features into
    xcat[t*128+p] = [src|dest|ea] so DMA streams are fully contiguous.
  * Device pipeline, all uniform (no per-core program specialization):
      val  = exp(leaky(TTR(xcat_tile, vcat)))     per 128-slot tile
      P    = segmented scan  state = m0*state + val          (forward)
      D    = P * islast
      tot  = propagation scan state = notlast*state + D      (reversed APs)
      out  = val * reciprocal(tot)
    Segment sums need no PSUM scatter / one-hots: two tensor_tensor_scan
    instructions replace them entirely.
"""

import math
import os
import sys
import time

import numpy as np

sys.path.insert(0, "/opt/trn_rl_repo")

P = 128
NCORES = 8
NEG_SLOPE = 0.2
IN = 128
ED = 32
F = IN + IN + ED  # 288
G = 8             # 128-slot tiles per DMA group

LAST_EXEC_NS = None
LAST_WALL_NS = None
LAST_RESULTS = None

_PROGRAM_CACHE = {}


# --------------------------------------------------------------------------- #
# Host-side preparation
# --------------------------------------------------------------------------- #

def _host_prep(col, n_nodes):
    """Sort edges by dest, carve per-core node ranges and per-core
    128 segment-aligned strips. Returns per-core slot metadata."""
    perm = np.argsort(col, kind="stable")
    col_s = col[perm]
    npc = math.ceil(n_nodes / NCORES)
    bounds = np.searchsorted(col_s, np.arange(NCORES + 1) * npc)

    cores = []
    T_req = 1
    for c in range(NCORES):
        lo, hi = int(bounds[c]), int(bounds[c + 1])
        n_c = hi - lo
        if n_c == 0:
            cores.append(dict(starts=np.full(P, lo), lens=np.zeros(P, np.int64)))
            continue
        seg = col_s[lo:hi]
        # positions (relative) where a new segment starts, excluding 0
        B = np.flatnonzero(np.diff(seg)) + 1
        ts = math.ceil(n_c / P)
        ideal = np.arange(1, P) * ts                      # [127]
        picks = np.searchsorted(B, ideal, side="left")
        starts_rel = np.concatenate(
            [[0], np.where(picks < len(B), B[np.minimum(picks, len(B) - 1)]
                           if len(B) else 0, n_c)])
        starts_rel = np.maximum.accumulate(starts_rel)
        ends_rel = np.concatenate([starts_rel[1:], [n_c]])
        lens = ends_rel - starts_rel
        cores.append(dict(starts=starts_rel + lo, lens=lens))
        T_req = max(T_req, int(lens.max()))

    T = math.ceil(T_req / G) * G
    S = P * T

    per_core = []
    tt = np.arange(T)
    E_tot = len(col)
    for c in range(NCORES):
        starts, lens = cores[c]["starts"], cores[c]["lens"]
        pos = starts[:, None] + tt[None, :]               # [P, T] sorted idx
        valid = tt[None, :] < lens[:, None]
        posc = np.minimum(pos, E_tot - 1)
        slot_edge = np.where(valid, perm[posc], -1)       # original edge id
        cs = col_s[posc]
        prev_same = np.zeros((P, T), bool)
        prev_same[:, 1:] = cs[:, 1:] == cs[:, :-1]
        m0 = (valid & prev_same).astype(np.float32)
        nxt = np.minimum(posc + 1, E_tot - 1)
        next_same = (col_s[nxt] == cs) & (tt[None, :] + 1 < lens[:, None])
        islast = (~next_same).astype(np.float32)          # pads: islast=1
        per_core.append(dict(slot_edge=slot_edge, m0=m0, islast=islast,
                             notlast=(1.0 - islast).astype(np.float32)))
    return per_core, T, S


def _build_xcat(slot_edge, src, dest, edge_attr, S):
    eid = slot_edge.T.reshape(-1)                         # row s = t*128+p
    xc = np.zeros((S, F), np.float32)
    m = eid >= 0
    idx = eid[m]
    xc[m, 0:IN] = src[idx]
    xc[m, IN:2 * IN] = dest[idx]
    xc[m, 2 * IN:] = edge_attr[idx]
    return xc


# --------------------------------------------------------------------------- #
# Device program (one SPMD program for all 8 cores)
# --------------------------------------------------------------------------- #

def _build_program(T):
    from concourse import bacc, dve_ops, mybir
    from concourse import tile

    f32 = mybir.dt.float32
    AF = mybir.ActivationFunctionType
    OP = mybir.AluOpType
    S = P * T
    assert T % G == 0

    nc = bacc.Bacc("TRN2", target_bir_lowering=False, debug=False)

    xcat = nc.declare_dram_parameter("xcat", [S, F], f32, isOutput=False)
    vcat = nc.declare_dram_parameter("vcat", [P, F], f32, isOutput=False)
    xm0 = nc.declare_dram_parameter("xm0", [P, T], f32, isOutput=False)
    xislast = nc.declare_dram_parameter("xislast", [P, T], f32, isOutput=False)
    xnotlast = nc.declare_dram_parameter("xnotlast", [P, T], f32,
                                         isOutput=False)
    yout = nc.declare_dram_parameter("yout", [P, T], f32, isOutput=True)

    with tile.TileContext(nc) as tc:
        with (
            tc.tile_pool(name="consts", bufs=1) as cpool,
            tc.tile_pool(name="stream", bufs=4) as spool,
            tc.tile_pool(name="scr", bufs=4) as rpool,
            tc.tile_pool(name="work", bufs=1) as wpool,
        ):
            vb = cpool.tile([P, F], f32, tag="vb")
            m0 = cpool.tile([P, T], f32, tag="m0")
            islast = cpool.tile([P, T], f32, tag="islast")
            notlast = cpool.tile([P, T], f32, tag="notlast")
            nc.sync.dma_start(out=vb[:], in_=vcat[:])
            nc.sync.dma_start(out=m0[:], in_=xm0[:])
            nc.sync.dma_start(out=islast[:], in_=xislast[:])
            nc.sync.dma_start(out=notlast[:], in_=xnotlast[:])

            val = wpool.tile([P, T], f32, tag="val")

            for b in range(T // G):
                xt = spool.tile([P, G, F], f32, tag="xt")
                src_ap = xcat[b * G * P:(b + 1) * G * P, :].rearrange(
                    "(g p) f -> p g f", p=P)
                nc.sync.dma_start(out=xt[:], in_=src_ap)
                scr = rpool.tile([P, F], f32, tag="scr")
                for g in range(G):
                    nc.vector._custom_dve(
                        dve_ops.TENSOR_TENSOR_REDUCE,
                        out=scr[:, :], in0=xt[:, g, :], in1=vb[:, :],
                        s0=0.0, s1=1.0,
                        accum_out=val[:, b * G + g:b * G + g + 1],
                    )

            # val = exp(leaky_relu(logits))
            tmp = wpool.tile([P, T], f32, tag="tmp")
            nc.vector.tensor_scalar(out=tmp[:, :], in0=val[:, :],
                                    scalar1=NEG_SLOPE, scalar2=None,
                                    op0=OP.mult)
            nc.vector.tensor_tensor(out=tmp[:, :], in0=val[:, :],
                                    in1=tmp[:, :], op=OP.max)
            nc.scalar.activation(val[:, :], tmp[:, :], AF.Exp)

            # forward segmented scan: within-segment running sum
            pseg = wpool.tile([P, T], f32, tag="pseg")
            nc.vector.tensor_tensor_scan(
                out=pseg[:, :], data0=m0[:, :], data1=val[:, :],
                initial=0.0, op0=OP.mult, op1=OP.add)

            # segment totals live at segment-last slots
            dlast = wpool.tile([P, T], f32, tag="dlast")
            nc.vector.tensor_tensor(out=dlast[:, :], in0=pseg[:, :],
                                    in1=islast[:, :], op=OP.mult)

            # propagate totals right-to-left across each segment
            segtot = wpool.tile([P, T], f32, tag="segtot")
            nc.vector.tensor_tensor_scan(
                out=segtot[:, ::-1], data0=notlast[:, ::-1],
                data1=dlast[:, ::-1],
                initial=0.0, op0=OP.mult, op1=OP.add)

            inv = wpool.tile([P, T], f32, tag="inv")
            nc.vector.reciprocal(inv[:, :], segtot[:, :])
            outv = wpool.tile([P, T], f32, tag="outv")
            nc.vector.tensor_tensor(out=outv[:, :], in0=val[:, :],
                                    in1=inv[:, :], op=OP.mult)
            nc.sync.dma_start(out=yout[:], in_=outv[:, :])

    nc.compile()
    return nc


# --------------------------------------------------------------------------- #
# Entry point
# --------------------------------------------------------------------------- #

def _ensure_ntff_hook():
    """Register the axon NTFF profiling hook if the image's antenv package
    lacks the axon_hooks module (boot degrades silently without it)."""
    import types

    try:
        from antenv import axon_hooks  # noqa: F401
    except ImportError:
        import antenv

        mod = types.ModuleType("antenv.axon_hooks")
        mod._hook = None
        mod.set_axon_ntff_profile_hook = lambda h: setattr(mod, "_hook", h)
        mod.get_axon_ntff_profile_hook = lambda: mod._hook
        sys.modules["antenv.axon_hooks"] = mod
        antenv.axon_hooks = mod
    from antenv.axon_hooks import (get_axon_ntff_profile_hook,
                                   set_axon_ntff_profile_hook)

    if get_axon_ntff_profile_hook() is None:
        from trn_agent_boot.trn_boot import _ntff_profile_via_ctypes

        h = _ntff_profile_via_ctypes("/opt/axon/libaxon_pjrt.so")
        if h is not None:
            set_axon_ntff_profile_hook(h)
    return get_axon_ntff_profile_hook()


def _run(nc, in_maps, trace):
    """Execute the SPMD program; optionally capture NTFF profiles and
    return (results, max_core_exec_ns, perfetto_results)."""
    import glob
    import tempfile

    from concourse import bass2jax

    if not trace:
        return bass2jax.run_bass_via_pjrt(nc, in_maps, n_cores=NCORES), None, None

    hook = None
    try:
        hook = _ensure_ntff_hook()
    except Exception as e:
        print(f"ntff hook unavailable: {e}")
    if hook is None:
        return bass2jax.run_bass_via_pjrt(nc, in_maps, n_cores=NCORES), None, None

    tmpdir = tempfile.mkdtemp(prefix="gnn_ntff_")
    with hook(tmpdir, list(range(NCORES))):
        results = bass2jax.run_bass_via_pjrt(nc, in_maps, n_cores=NCORES)

    ntffs = glob.glob(os.path.join(tmpdir, "*_body*.ntff"))
    if not ntffs:
        print(f"no NTFFs captured in {tmpdir}")
        return results, None, None

    import gauge.profiler
    from concourse._compat import FishPath

    profile = gauge.profiler.Profile(
        profile_path=FishPath(tmpdir), kernel_dev_mode=True,
        profile_on_exit=False, bass_kernel=nc.m, offline_processing=True,
        fname="*_body*", metadata={})
    pr = profile.to_perfetto(model_index=tuple(range(NCORES)))
    exec_ns = max(r.exec_time_ns for r in pr) if pr else None
    return results, exec_ns, pr


def kernel(src, dest, edge_attr, edge_index, n_nodes,
           W_src, W_dest, W_edge, attn_vector):
    global LAST_EXEC_NS, LAST_WALL_NS, LAST_RESULTS

    src = np.asarray(src, np.float32)
    dest = np.asarray(dest, np.float32)
    edge_attr = np.asarray(edge_attr, np.float32)
    edge_index = np.asarray(edge_index)
    N = int(n_nodes)
    E = src.shape[0]

    a = np.asarray(attn_vector, np.float32)[0]
    vcat_row = np.concatenate([
        np.asarray(W_src, np.float32) @ a,
        np.asarray(W_dest, np.float32) @ a,
        np.asarray(W_edge, np.float32) @ a]).astype(np.float32)
    vcat = np.broadcast_to(vcat_row, (P, F)).copy()

    col = edge_index[1].astype(np.int64)
    per_core, T, S = _host_prep(col, N)

    if T not in _PROGRAM_CACHE:
        _PROGRAM_CACHE[T] = _build_program(T)
    nc = _PROGRAM_CACHE[T]

    in_maps = []
    for c in range(NCORES):
        pc = per_core[c]
        in_maps.append(dict(
            xcat=_build_xcat(pc["slot_edge"], src, dest, edge_attr, S),
            vcat=vcat, xm0=pc["m0"], xislast=pc["islast"],
            xnotlast=pc["notlast"],
        ))

    trace = bool(os.environ.get("KPROFILE"))
    t0 = time.perf_counter_ns()
    results, exec_ns, pr = _run(nc, in_maps, trace)
    LAST_WALL_NS = time.perf_counter_ns() - t0
    LAST_EXEC_NS = exec_ns
    LAST_RESULTS = pr

    out_full = np.zeros((E,), np.float32)
    for c in range(NCORES):
        y = results[c]["yout"]                            # [P, T]
        se = per_core[c]["slot_edge"]
        m = se >= 0
        out_full[se[m]] = y[m]
    return out_full[:, None]


# revision 8
# speedup vs baseline: 1668.0530x; 1.0422x over previous
"""GNN edge segment-softmax attention kernel for 8 Trainium2 NeuronCores.

Math: logits = src@(W_src@a) + dest@(W_dest@a) + ea@(W_edge@a)    [E]
      s = leaky_relu(logits, 0.2); val = exp(s)
      out[e] = val[e] / segsum[col[e]]      (softmax over dest node; the
      reference's eps=1e-16 is negligible: every segsum >= exp(-|s|max) >> eps)

Strategy (single SPMD program on 8 cores, memory-roofline streaming):
  * Fold projection matrices with the attention vector on host: per-edge
    work becomes one 288-wide dot product. Features ship as fp16 (halves
    HBM traffic; logit error ~1e-4 << the 2e-2 gate).
  * Host sorts edges by destination node; core c owns nodes
    [c*N/8, (c+1)*N/8) so every softmax segment is core-local.
  * Within a core, sorted edges are split into 128 partition-strips, each
    strip aligned to segment boundaries (a node's edges never cross strips).
    Slot (p, t): partition p, position t; host gathers features into
    xcat[t*128+p] = [src|dest|ea] so DMA streams are fully contiguous.
  * Device pipeline, all uniform (no per-core program specialization):
      val  = exp(leaky(ttr(xcat_tile, vcat)))   native fused DVE dot/tile
      P    = segmented scan  state = m0*state + val          (forward)
      D    = P * islast
      tot  = propagation scan state = notlast*state + D      (reversed APs)
      out  = val * reciprocal(tot)
    Segment sums need no PSUM scatter / one-hots: two tensor_tensor_scan
    instructions replace them entirely. islast/notlast derive from m0
    on device (shifted views), so only one small mask streams in.
"""

import math
import os
import sys
import time

import numpy as np

sys.path.insert(0, "/opt/trn_rl_repo")

P = 128
NCORES = 8
NEG_SLOPE = 0.2
IN = 128
ED = 32
F = IN + IN + ED  # 288
G = 16            # 128-slot tiles per DMA group

LAST_EXEC_NS = None
LAST_WALL_NS = None
LAST_RESULTS = None
LAST_T = None

_PROGRAM_CACHE = {}


# --------------------------------------------------------------------------- #
# Host-side preparation
# --------------------------------------------------------------------------- #

def _host_prep(col, n_nodes):
    """Sort edges by dest, carve per-core node ranges and per-core
    128 segment-aligned strips. Returns per-core slot metadata."""
    perm = np.argsort(col, kind="stable")
    col_s = col[perm]
    npc = math.ceil(n_nodes / NCORES)
    bounds = np.searchsorted(col_s, np.arange(NCORES + 1) * npc)

    cores = []
    T_req = 1
    for c in range(NCORES):
        lo, hi = int(bounds[c]), int(bounds[c + 1])
        n_c = hi - lo
        if n_c == 0:
            cores.append(dict(starts=np.full(P, lo), lens=np.zeros(P, np.int64)))
            continue
        seg = col_s[lo:hi]
        # positions (relative) where a new segment starts, excluding 0
        B = np.flatnonzero(np.diff(seg)) + 1
        ts = n_c / P
        ideal = np.arange(1, P) * ts                      # [127]
        if len(B):
            picks = np.searchsorted(B, ideal, side="left")
            # nearest boundary to the ideal split (balance strip lengths)
            lo_pick = np.maximum(picks - 1, 0)
            hi_pick = np.minimum(picks, len(B) - 1)
            use_hi = (np.abs(B[hi_pick] - ideal)
                      <= np.abs(B[lo_pick] - ideal)) & (picks < len(B))
            chosen = np.where(use_hi, B[hi_pick], B[lo_pick])
            chosen = np.where(picks == 0, B[hi_pick], chosen)
            starts_rel = np.concatenate([[0], chosen])
        else:
            starts_rel = np.concatenate([[0], np.full(P - 1, n_c)])
        starts_rel = np.maximum.accumulate(starts_rel)
        ends_rel = np.concatenate([starts_rel[1:], [n_c]])
        lens = ends_rel - starts_rel
        cores.append(dict(starts=starts_rel + lo, lens=lens))
        T_req = max(T_req, int(lens.max()))

    T = math.ceil(T_req / G) * G
    S = P * T

    per_core = []
    tt = np.arange(T)
    E_tot = len(col)
    for c in range(NCORES):
        starts, lens = cores[c]["starts"], cores[c]["lens"]
        pos = starts[:, None] + tt[None, :]               # [P, T] sorted idx
        valid = tt[None, :] < lens[:, None]
        posc = np.minimum(pos, E_tot - 1)
        slot_edge = np.where(valid, perm[posc], -1)       # original edge id
        cs = col_s[posc]
        prev_same = np.zeros((P, T), bool)
        prev_same[:, 1:] = cs[:, 1:] == cs[:, :-1]
        m0 = (valid & prev_same).astype(np.float16)
        per_core.append(dict(slot_edge=slot_edge, m0=m0))
    return per_core, T, S


def _build_xcat(slot_edge, src, dest, edge_attr, S):
    eid = slot_edge.T.reshape(-1)                         # row s = t*128+p
    xc = np.zeros((S, F), np.float16)
    m = eid >= 0
    idx = eid[m]
    xc[m, 0:IN] = src[idx]
    xc[m, IN:2 * IN] = dest[idx]
    xc[m, 2 * IN:] = edge_attr[idx]
    return xc


# --------------------------------------------------------------------------- #
# Device program (one SPMD program for all 8 cores)
# --------------------------------------------------------------------------- #

def _build_program(T, use_native=False, f16_on=True):
    from concourse import bacc, dve_ops, mybir
    from concourse import tile

    f32 = mybir.dt.float32
    f16 = mybir.dt.float16 if f16_on else mybir.dt.float32
    AF = mybir.ActivationFunctionType
    OP = mybir.AluOpType
    S = P * T
    assert T % G == 0

    nc = bacc.Bacc("TRN2", target_bir_lowering=False, debug=False)

    xcat = nc.declare_dram_parameter("xcat", [S, F], f16, isOutput=False)
    vcat = nc.declare_dram_parameter("vcat", [P, F], f16, isOutput=False)
    xm0 = nc.declare_dram_parameter("xm0", [P, T], f16, isOutput=False)
    yout = nc.declare_dram_parameter("yout", [P, T], f32, isOutput=True)

    with tile.TileContext(nc) as tc:
        with (
            tc.tile_pool(name="consts", bufs=1) as cpool,
            tc.tile_pool(name="stream", bufs=4) as spool,
            tc.tile_pool(name="scr", bufs=4) as rpool,
            tc.tile_pool(name="work", bufs=1) as wpool,
        ):
            vb = cpool.tile([P, F], f16, tag="vb")
            m0 = cpool.tile([P, T], f16, tag="m0")
            nc.sync.dma_start(out=vb[:], in_=vcat[:])
            nc.sync.dma_start(out=m0[:], in_=xm0[:])

            val = wpool.tile([P, T], f32, tag="val")

            for b in range(T // G):
                xt = spool.tile([P, G, F], f16, tag="xt")
                src_ap = xcat[b * G * P:(b + 1) * G * P, :].rearrange(
                    "(g p) f -> p g f", p=P)
                nc.sync.dma_start(out=xt[:], in_=src_ap)
                scr = rpool.tile([P, F], f16, tag="scr")
                for g in range(G):
                    if use_native:
                        nc.vector.tensor_tensor_reduce(
                            out=scr[:, :], in0=xt[:, g, :], in1=vb[:, :],
                            scale=1.0, scalar=0.0, op0=OP.mult, op1=OP.add,
                            accum_out=val[:, b * G + g:b * G + g + 1],
                        )
                    else:
                        nc.vector._custom_dve(
                            dve_ops.TENSOR_TENSOR_REDUCE,
                            out=scr[:, :], in0=xt[:, g, :], in1=vb[:, :],
                            s0=0.0, s1=1.0,
                            accum_out=val[:, b * G + g:b * G + g + 1],
                        )

            # masks: m0f = fp32 m0; notlast[t] = m0[t+1]; islast = 1-notlast
            m0f = wpool.tile([P, T], f32, tag="m0f")
            nc.vector.tensor_scalar(out=m0f[:, :], in0=m0[:, :],
                                    scalar1=1.0, scalar2=None, op0=OP.mult)
            nl = wpool.tile([P, T], f32, tag="nl")
            nc.vector.memset(nl[:, T - 1:T], 0.0)
            nc.vector.tensor_scalar(out=nl[:, 0:T - 1], in0=m0[:, 1:T],
                                    scalar1=1.0, scalar2=None, op0=OP.mult)
            il = wpool.tile([P, T], f32, tag="il")
            nc.vector.tensor_scalar(out=il[:, :], in0=nl[:, :],
                                    scalar1=-1.0, scalar2=1.0,
                                    op0=OP.mult, op1=OP.add)

            # val = exp(leaky_relu(logits))
            tmp = wpool.tile([P, T], f32, tag="tmp")
            nc.vector.tensor_scalar(out=tmp[:, :], in0=val[:, :],
                                    scalar1=NEG_SLOPE, scalar2=None,
                                    op0=OP.mult)
            nc.vector.tensor_tensor(out=tmp[:, :], in0=val[:, :],
                                    in1=tmp[:, :], op=OP.max)
            nc.scalar.activation(val[:, :], tmp[:, :], AF.Exp)

            # forward segmented scan: within-segment running sum
            pseg = wpool.tile([P, T], f32, tag="pseg")
            nc.vector.tensor_tensor_scan(
                out=pseg[:, :], data0=m0f[:, :], data1=val[:, :],
                initial=0.0, op0=OP.mult, op1=OP.add)

            # segment totals live at segment-last slots
            dlast = wpool.tile([P, T], f32, tag="dlast")
            nc.vector.tensor_tensor(out=dlast[:, :], in0=pseg[:, :],
                                    in1=il[:, :], op=OP.mult)

            # propagate totals right-to-left across each segment
            segtot = wpool.tile([P, T], f32, tag="segtot")
            nc.vector.tensor_tensor_scan(
                out=segtot[:, ::-1], data0=nl[:, ::-1],
                data1=dlast[:, ::-1],
                initial=0.0, op0=OP.mult, op1=OP.add)

            inv = wpool.tile([P, T], f32, tag="inv")
            nc.vector.reciprocal(inv[:, :], segtot[:, :])
            outv = wpool.tile([P, T], f32, tag="outv")
            nc.vector.tensor_tensor(out=outv[:, :], in0=val[:, :],
                                    in1=inv[:, :], op=OP.mult)
            nc.sync.dma_start(out=yout[:], in_=outv[:, :])

    nc.compile()
    return nc


# --------------------------------------------------------------------------- #
# Execution helpers
# --------------------------------------------------------------------------- #

def _ensure_ntff_hook():
    """Register the axon NTFF profiling hook if the image's antenv package
    lacks the axon_hooks module (boot degrades silently without it)."""
    import types

    try:
        from antenv import axon_hooks  # noqa: F401
    except ImportError:
        import antenv

        mod = types.ModuleType("antenv.axon_hooks")
        mod._hook = None
        mod.set_axon_ntff_profile_hook = lambda h: setattr(mod, "_hook", h)
        mod.get_axon_ntff_profile_hook = lambda: mod._hook
        sys.modules["antenv.axon_hooks"] = mod
        antenv.axon_hooks = mod
    from antenv.axon_hooks import (get_axon_ntff_profile_hook,
                                   set_axon_ntff_profile_hook)

    if get_axon_ntff_profile_hook() is None:
        from trn_agent_boot.trn_boot import _ntff_profile_via_ctypes

        h = _ntff_profile_via_ctypes("/opt/axon/libaxon_pjrt.so")
        if h is not None:
            set_axon_ntff_profile_hook(h)
    return get_axon_ntff_profile_hook()


def _run(nc, in_maps, trace):
    """Execute the SPMD program; optionally capture NTFF profiles and
    return (results, max_core_exec_ns, perfetto_results)."""
    import glob
    import tempfile

    from concourse import bass2jax

    if not trace:
        return bass2jax.run_bass_via_pjrt(nc, in_maps, n_cores=NCORES), None, None

    hook = None
    try:
        hook = _ensure_ntff_hook()
    except Exception as e:
        print(f"ntff hook unavailable: {e}")
    if hook is None:
        return bass2jax.run_bass_via_pjrt(nc, in_maps, n_cores=NCORES), None, None

    tmpdir = tempfile.mkdtemp(prefix="gnn_ntff_")
    with hook(tmpdir, list(range(NCORES))):
        results = bass2jax.run_bass_via_pjrt(nc, in_maps, n_cores=NCORES)

    ntffs = glob.glob(os.path.join(tmpdir, "*_body*.ntff"))
    if not ntffs:
        print(f"no NTFFs captured in {tmpdir}")
        return results, None, None

    import gauge.profiler
    from concourse._compat import FishPath

    profile = gauge.profiler.Profile(
        profile_path=FishPath(tmpdir), kernel_dev_mode=True,
        profile_on_exit=False, bass_kernel=nc.m, offline_processing=True,
        fname="*_body*", metadata={})
    pr = profile.to_perfetto(model_index=tuple(range(NCORES)))
    exec_ns = max(r.exec_time_ns for r in pr) if pr else None
    return results, exec_ns, pr


# --------------------------------------------------------------------------- #
# Entry point
# --------------------------------------------------------------------------- #

def kernel(src, dest, edge_attr, edge_index, n_nodes,
           W_src, W_dest, W_edge, attn_vector):
    global LAST_EXEC_NS, LAST_WALL_NS, LAST_RESULTS, LAST_T

    src = np.asarray(src, np.float32)
    dest = np.asarray(dest, np.float32)
    edge_attr = np.asarray(edge_attr, np.float32)
    edge_index = np.asarray(edge_index)
    N = int(n_nodes)
    E = src.shape[0]

    a = np.asarray(attn_vector, np.float32)[0]
    vcat_row = np.concatenate([
        np.asarray(W_src, np.float32) @ a,
        np.asarray(W_dest, np.float32) @ a,
        np.asarray(W_edge, np.float32) @ a]).astype(np.float16)
    vcat = np.broadcast_to(vcat_row, (P, F)).copy()

    col = edge_index[1].astype(np.int64)
    per_core, T, S = _host_prep(col, N)
    LAST_T = T

    if T not in _PROGRAM_CACHE:
        _PROGRAM_CACHE[T] = _build_program(T)
    nc = _PROGRAM_CACHE[T]

    src16 = src.astype(np.float16)
    dest16 = dest.astype(np.float16)
    ea16 = edge_attr.astype(np.float16)
    in_maps = []
    for c in range(NCORES):
        pc = per_core[c]
        in_maps.append(dict(
            xcat=_build_xcat(pc["slot_edge"], src16, dest16, ea16, S),
            vcat=vcat, xm0=pc["m0"],
        ))

    trace = bool(os.environ.get("KPROFILE"))
    t0 = time.perf_counter_ns()
    results, exec_ns, pr = _run(nc, in_maps, trace)
    LAST_WALL_NS = time.perf_counter_ns() - t0
    LAST_EXEC_NS = exec_ns
    LAST_RESULTS = pr

    out_full = np.zeros((E,), np.float32)
    for c in range(NCORES):
        y = results[c]["yout"]                            # [P, T]
        se = per_core[c]["slot_edge"]
        m = se >= 0
        out_full[se[m]] = y[m]
    return out_full[:, None]


# revision 10
# speedup vs baseline: 1692.8878x; 1.0149x over previous
"""GNN edge segment-softmax attention kernel for 8 Trainium2 NeuronCores.

Math: logits = src@(W_src@a) + dest@(W_dest@a) + ea@(W_edge@a)    [E]
      s = leaky_relu(logits, 0.2); val = exp(s)
      out[e] = val[e] / segsum[col[e]]      (softmax over dest node; the
      reference's eps=1e-16 is negligible: every segsum >= exp(-|s|max) >> eps)

Strategy (single SPMD program on 8 cores, memory-roofline streaming):
  * Fold projection matrices with the attention vector on host: per-edge
    work becomes one 288-wide dot product. Features ship as fp16 (halves
    HBM traffic; logit error ~1e-4 << the 2e-2 gate).
  * Host sorts edges by destination node; core c owns nodes
    [c*N/8, (c+1)*N/8) so every softmax segment is core-local.
  * Within a core, sorted edges are split into 128 partition-strips, each
    strip aligned to segment boundaries (a node's edges never cross strips).
    Slot (p, t): partition p, position t; host gathers features into
    xcat[t*128+p] = [src|dest|ea] so DMA streams are fully contiguous.
  * Device pipeline, all uniform (no per-core program specialization):
      val  = exp(leaky(ttr(xcat_tile, vcat)))   native fused DVE dot/tile
      P    = segmented scan  state = m0*state + val          (forward)
      D    = P * islast
      tot  = propagation scan state = notlast*state + D      (reversed APs)
      out  = val * reciprocal(tot)
    Segment sums need no PSUM scatter / one-hots: two tensor_tensor_scan
    instructions replace them entirely. islast/notlast derive from m0
    on device (shifted views), so only one small mask streams in.
"""

import math
import os
import sys
import time

import numpy as np

sys.path.insert(0, "/opt/trn_rl_repo")

P = 128
NCORES = 8
NEG_SLOPE = 0.2
IN = 128
ED = 32
F = IN + IN + ED  # 288
G = 16            # 128-slot tiles per DMA group

LAST_EXEC_NS = None
LAST_WALL_NS = None
LAST_RESULTS = None
LAST_T = None

_PROGRAM_CACHE = {}


# --------------------------------------------------------------------------- #
# Host-side preparation
# --------------------------------------------------------------------------- #

def _host_prep(col, n_nodes):
    """Sort edges by dest, carve per-core node ranges and per-core
    128 segment-aligned strips. Returns per-core slot metadata."""
    perm = np.argsort(col, kind="stable")
    col_s = col[perm]
    npc = math.ceil(n_nodes / NCORES)
    bounds = np.searchsorted(col_s, np.arange(NCORES + 1) * npc)

    cores = []
    T_req = 1
    for c in range(NCORES):
        lo, hi = int(bounds[c]), int(bounds[c + 1])
        n_c = hi - lo
        if n_c == 0:
            cores.append(dict(starts=np.full(P, lo), lens=np.zeros(P, np.int64)))
            continue
        seg = col_s[lo:hi]
        # positions (relative) where a new segment starts, excluding 0
        B = np.flatnonzero(np.diff(seg)) + 1
        ts = n_c / P
        ideal = np.arange(1, P) * ts                      # [127]
        if len(B):
            picks = np.searchsorted(B, ideal, side="left")
            # nearest boundary to the ideal split (balance strip lengths)
            lo_pick = np.maximum(picks - 1, 0)
            hi_pick = np.minimum(picks, len(B) - 1)
            use_hi = (np.abs(B[hi_pick] - ideal)
                      <= np.abs(B[lo_pick] - ideal)) & (picks < len(B))
            chosen = np.where(use_hi, B[hi_pick], B[lo_pick])
            chosen = np.where(picks == 0, B[hi_pick], chosen)
            starts_rel = np.concatenate([[0], chosen])
        else:
            starts_rel = np.concatenate([[0], np.full(P - 1, n_c)])
        starts_rel = np.maximum.accumulate(starts_rel)
        ends_rel = np.concatenate([starts_rel[1:], [n_c]])
        lens = ends_rel - starts_rel
        cores.append(dict(starts=starts_rel + lo, lens=lens))
        T_req = max(T_req, int(lens.max()))

    T = math.ceil(T_req / G) * G
    S = P * T

    per_core = []
    tt = np.arange(T)
    E_tot = len(col)
    for c in range(NCORES):
        starts, lens = cores[c]["starts"], cores[c]["lens"]
        pos = starts[:, None] + tt[None, :]               # [P, T] sorted idx
        valid = tt[None, :] < lens[:, None]
        posc = np.minimum(pos, E_tot - 1)
        slot_edge = np.where(valid, perm[posc], -1)       # original edge id
        cs = col_s[posc]
        prev_same = np.zeros((P, T), bool)
        prev_same[:, 1:] = cs[:, 1:] == cs[:, :-1]
        m0 = (valid & prev_same).astype(np.float16)
        per_core.append(dict(slot_edge=slot_edge, m0=m0))
    return per_core, T, S


def _build_xcat(slot_edge, src, dest, edge_attr, S):
    eid = slot_edge.reshape(-1)                           # row s = p*T+t
    xc = np.zeros((S, F), np.float16)
    m = eid >= 0
    idx = eid[m]
    xc[m, 0:IN] = src[idx]
    xc[m, IN:2 * IN] = dest[idx]
    xc[m, 2 * IN:] = edge_attr[idx]
    return xc


# --------------------------------------------------------------------------- #
# Device program (one SPMD program for all 8 cores)
# --------------------------------------------------------------------------- #

def _build_program(T, use_native=False, f16_on=True):
    from concourse import bacc, dve_ops, mybir
    from concourse import tile

    f32 = mybir.dt.float32
    f16 = mybir.dt.float16 if f16_on else mybir.dt.float32
    AF = mybir.ActivationFunctionType
    OP = mybir.AluOpType
    S = P * T
    assert T % G == 0

    nc = bacc.Bacc("TRN2", target_bir_lowering=False, debug=False)

    xcat = nc.declare_dram_parameter("xcat", [S, F], f16, isOutput=False)
    vcat = nc.declare_dram_parameter("vcat", [P, F], f16, isOutput=False)
    xm0 = nc.declare_dram_parameter("xm0", [P, T], f16, isOutput=False)
    yout = nc.declare_dram_parameter("yout", [P, T], f32, isOutput=True)

    with tile.TileContext(nc) as tc:
        with (
            tc.tile_pool(name="consts", bufs=1) as cpool,
            tc.tile_pool(name="stream", bufs=4) as spool,
            tc.tile_pool(name="scr", bufs=4) as rpool,
            tc.tile_pool(name="work", bufs=1) as wpool,
        ):
            vb = cpool.tile([P, F], f16, tag="vb")
            m0 = cpool.tile([P, T], f16, tag="m0")
            nc.sync.dma_start(out=vb[:], in_=vcat[:])
            nc.sync.dma_start(out=m0[:], in_=xm0[:])

            val = wpool.tile([P, T], f32, tag="val")

            # xcat rows are p-major (slot s = p*T + t): partition p's group-b
            # data is ONE contiguous G*F*2B run -> near-peak DMA efficiency.
            xview = xcat.rearrange("(p t) f -> p t f", p=P)
            for b in range(T // G):
                xt = spool.tile([P, G, F], f16, tag="xt")
                nc.sync.dma_start(out=xt[:], in_=xview[:, b * G:(b + 1) * G, :])
                prod = rpool.tile([P, G, F], f16, tag="prod")
                nc.vector.tensor_tensor(
                    out=prod[:], in0=xt[:],
                    in1=vb[:].unsqueeze(1).broadcast_to([P, G, F]),
                    op=OP.mult)
                nc.vector.tensor_reduce(
                    out=val[:, b * G:(b + 1) * G], in_=prod[:],
                    axis=mybir.AxisListType.X, op=OP.add)

            # masks: m0f = fp32 m0; notlast[t] = m0[t+1]; islast = 1-notlast
            m0f = wpool.tile([P, T], f32, tag="m0f")
            nc.vector.tensor_scalar(out=m0f[:, :], in0=m0[:, :],
                                    scalar1=1.0, scalar2=None, op0=OP.mult)
            nl = wpool.tile([P, T], f32, tag="nl")
            nc.vector.memset(nl[:, T - 1:T], 0.0)
            nc.vector.tensor_scalar(out=nl[:, 0:T - 1], in0=m0[:, 1:T],
                                    scalar1=1.0, scalar2=None, op0=OP.mult)
            il = wpool.tile([P, T], f32, tag="il")
            nc.vector.tensor_scalar(out=il[:, :], in0=nl[:, :],
                                    scalar1=-1.0, scalar2=1.0,
                                    op0=OP.mult, op1=OP.add)

            # val = exp(leaky_relu(logits))
            tmp = wpool.tile([P, T], f32, tag="tmp")
            nc.vector.tensor_scalar(out=tmp[:, :], in0=val[:, :],
                                    scalar1=NEG_SLOPE, scalar2=None,
                                    op0=OP.mult)
            nc.vector.tensor_tensor(out=tmp[:, :], in0=val[:, :],
                                    in1=tmp[:, :], op=OP.max)
            nc.scalar.activation(val[:, :], tmp[:, :], AF.Exp)

            # forward segmented scan: within-segment running sum
            pseg = wpool.tile([P, T], f32, tag="pseg")
            nc.vector.tensor_tensor_scan(
                out=pseg[:, :], data0=m0f[:, :], data1=val[:, :],
                initial=0.0, op0=OP.mult, op1=OP.add)

            # segment totals live at segment-last slots
            dlast = wpool.tile([P, T], f32, tag="dlast")
            nc.vector.tensor_tensor(out=dlast[:, :], in0=pseg[:, :],
                                    in1=il[:, :], op=OP.mult)

            # propagate totals right-to-left across each segment
            segtot = wpool.tile([P, T], f32, tag="segtot")
            nc.vector.tensor_tensor_scan(
                out=segtot[:, ::-1], data0=nl[:, ::-1],
                data1=dlast[:, ::-1],
                initial=0.0, op0=OP.mult, op1=OP.add)

            inv = wpool.tile([P, T], f32, tag="inv")
            nc.vector.reciprocal(inv[:, :], segtot[:, :])
            outv = wpool.tile([P, T], f32, tag="outv")
            nc.vector.tensor_tensor(out=outv[:, :], in0=val[:, :],
                                    in1=inv[:, :], op=OP.mult)
            nc.sync.dma_start(out=yout[:], in_=outv[:, :])

    nc.compile()
    return nc


# --------------------------------------------------------------------------- #
# Execution helpers
# --------------------------------------------------------------------------- #

def _ensure_ntff_hook():
    """Register the axon NTFF profiling hook if the image's antenv package
    lacks the axon_hooks module (boot degrades silently without it)."""
    import types

    try:
        from antenv import axon_hooks  # noqa: F401
    except ImportError:
        import antenv

        mod = types.ModuleType("antenv.axon_hooks")
        mod._hook = None
        mod.set_axon_ntff_profile_hook = lambda h: setattr(mod, "_hook", h)
        mod.get_axon_ntff_profile_hook = lambda: mod._hook
        sys.modules["antenv.axon_hooks"] = mod
        antenv.axon_hooks = mod
    from antenv.axon_hooks import (get_axon_ntff_profile_hook,
                                   set_axon_ntff_profile_hook)

    if get_axon_ntff_profile_hook() is None:
        from trn_agent_boot.trn_boot import _ntff_profile_via_ctypes

        h = _ntff_profile_via_ctypes("/opt/axon/libaxon_pjrt.so")
        if h is not None:
            set_axon_ntff_profile_hook(h)
    return get_axon_ntff_profile_hook()


def _run(nc, in_maps, trace):
    """Execute the SPMD program; optionally capture NTFF profiles and
    return (results, max_core_exec_ns, perfetto_results)."""
    import glob
    import tempfile

    from concourse import bass2jax

    if not trace:
        return bass2jax.run_bass_via_pjrt(nc, in_maps, n_cores=NCORES), None, None

    hook = None
    try:
        hook = _ensure_ntff_hook()
    except Exception as e:
        print(f"ntff hook unavailable: {e}")
    if hook is None:
        return bass2jax.run_bass_via_pjrt(nc, in_maps, n_cores=NCORES), None, None

    tmpdir = tempfile.mkdtemp(prefix="gnn_ntff_")
    with hook(tmpdir, list(range(NCORES))):
        results = bass2jax.run_bass_via_pjrt(nc, in_maps, n_cores=NCORES)

    ntffs = glob.glob(os.path.join(tmpdir, "*_body*.ntff"))
    if not ntffs:
        print(f"no NTFFs captured in {tmpdir}")
        return results, None, None

    import gauge.profiler
    from concourse._compat import FishPath

    profile = gauge.profiler.Profile(
        profile_path=FishPath(tmpdir), kernel_dev_mode=True,
        profile_on_exit=False, bass_kernel=nc.m, offline_processing=True,
        fname="*_body*", metadata={})
    pr = profile.to_perfetto(model_index=tuple(range(NCORES)))
    exec_ns = max(r.exec_time_ns for r in pr) if pr else None
    return results, exec_ns, pr


# --------------------------------------------------------------------------- #
# Entry point
# --------------------------------------------------------------------------- #

def kernel(src, dest, edge_attr, edge_index, n_nodes,
           W_src, W_dest, W_edge, attn_vector):
    global LAST_EXEC_NS, LAST_WALL_NS, LAST_RESULTS, LAST_T

    src = np.asarray(src, np.float32)
    dest = np.asarray(dest, np.float32)
    edge_attr = np.asarray(edge_attr, np.float32)
    edge_index = np.asarray(edge_index)
    N = int(n_nodes)
    E = src.shape[0]

    a = np.asarray(attn_vector, np.float32)[0]
    vcat_row = np.concatenate([
        np.asarray(W_src, np.float32) @ a,
        np.asarray(W_dest, np.float32) @ a,
        np.asarray(W_edge, np.float32) @ a]).astype(np.float16)
    vcat = np.broadcast_to(vcat_row, (P, F)).copy()

    col = edge_index[1].astype(np.int64)
    per_core, T, S = _host_prep(col, N)
    LAST_T = T

    if T not in _PROGRAM_CACHE:
        _PROGRAM_CACHE[T] = _build_program(T)
    nc = _PROGRAM_CACHE[T]

    src16 = src.astype(np.float16)
    dest16 = dest.astype(np.float16)
    ea16 = edge_attr.astype(np.float16)
    in_maps = []
    for c in range(NCORES):
        pc = per_core[c]
        in_maps.append(dict(
            xcat=_build_xcat(pc["slot_edge"], src16, dest16, ea16, S),
            vcat=vcat, xm0=pc["m0"],
        ))

    trace = bool(os.environ.get("KPROFILE"))
    t0 = time.perf_counter_ns()
    results, exec_ns, pr = _run(nc, in_maps, trace)
    LAST_WALL_NS = time.perf_counter_ns() - t0
    LAST_EXEC_NS = exec_ns
    LAST_RESULTS = pr

    out_full = np.zeros((E,), np.float32)
    for c in range(NCORES):
        y = results[c]["yout"]                            # [P, T]
        se = per_core[c]["slot_edge"]
        m = se >= 0
        out_full[se[m]] = y[m]
    return out_full[:, None]
